# revision 29
# baseline (speedup 1.0000x reference)
"""Trainium2 Bass kernel for nearest-neighbor descriptor matching (retrieval_knn).

Problem: desc1 (8192, 128), desc2 (8192, 128) fp32. For each row of desc1 find
the nearest row of desc2 under Euclidean distance. Returns
(match_dists (8192, 1) f32, matches_idxs (8192, 2) int32) like the reference.

Strategy (data-parallel over desc1 rows, 8 NeuronCores):
  - each core owns a 1024-row strip of desc1 and a full replica of desc2
  - distances via the norm expansion: d2 = sq1[m] + sq2[n] - 2 * <a_m, b_n>
  - per 128-row chunk (8 chunks): 16 matmuls (K=128, N=512) write -2*cross to
    PSUM; DVE adds the (pre-broadcast) sq2 row into a whole-chunk SBUF vals
    buffer and reduces each 512-wide tile to its min
  - row min via a reduce over the 16 tile-mins; argmin via one max_index
    scan of the row for the min value (first occurrence == np.argmin)
  - device returns clamped squared distances; the final elementwise
    np.sqrt happens on the host (correctly-rounded, matches the reference)

The engine-instruction encodings on this path carry at most ONE semaphore
wait each (including the kernel-tail drain, which waits once per busy
processor), so the kernel is shaped to minimize distinct processors:
one packed input DMA + one broadcast DMA + one packed output DMA, PE + DVE
only, with dummy clock-sync ops so no instruction ever needs two waits.

Inputs per core:
  dx [128, 9288] f32: cols 0..8191 desc2.T; 8192..9215 (-2*desc1 shard).T;
      9216..9223 sq1 in [p, chunk] layout; 9224..9287 sq2 reshaped (128, 64)
Output per core: out [2048] u32: first 1024 = clamped d2 bits (f32),
  last 1024 = argmin indices.

Self-contained: shapes/sharding hardcoded; no file reads.
"""

import os

import numpy as np

import concourse.bacc as bacc
import concourse.bass as bass
import concourse.mybir as mybir
import concourse.tile as tile
from concourse.bass_utils import run_bass_kernel_spmd

# ---- problem constants (hardcoded per contract) ----
B1, B2, D = 8192, 8192, 128
NCORES = 8
B1S = B1 // NCORES          # rows of desc1 per core = 1024
NCHUNK = B1S // 128         # 128-row chunks per core = 8
NT = B2 // 512              # 512-wide tiles per row = 16
TW = 512                    # tile width (one PSUM bank of fp32)
DX = B2 + B1S + NCHUNK + B2          # 8192 + 1024 + 8 + 8192 = 17416
EPS = 1e-12

F32 = mybir.dt.float32
U32 = mybir.dt.uint32

_BUILT = None  # cached Bass program


def _build():
    from contextlib import ExitStack

    nc = bacc.Bacc("TRN2", target_bir_lowering=False)

    dx = nc.dram_tensor("dx", [D, DX], F32, kind="ExternalInput")
    out_o = nc.dram_tensor("out", [2 * B1S], U32, kind="ExternalOutput")

    with ExitStack() as ctx:
        tc = ctx.enter_context(tile.TileContext(nc))
        const = ctx.enter_context(tc.tile_pool(name="const", bufs=1))
        small = ctx.enter_context(tc.tile_pool(name="small", bufs=8))
        valsp = ctx.enter_context(tc.tile_pool(name="valsp", bufs=2))
        psum = ctx.enter_context(tc.tile_pool(name="psum", bufs=8, space="PSUM"))
        outp = ctx.enter_context(tc.tile_pool(name="outp", bufs=1))

        # ---- prologue: one packed load + one broadcast load ----
        dx_sb = const.tile([D, DX], F32)
        nc.sync.dma_start(dx_sb[:], dx[:])

        d2t_sb = dx_sb[:, 0:B2]
        sq2b = dx_sb[:, B2 + B1S + NCHUNK:]
        d1t_sb = dx_sb[:, B2:B2 + B1S]
        sq1_sb = dx_sb[:, B2 + B1S:B2 + B1S + NCHUNK]

        # PE clock sync past the packed-input DMA (matmuls carry 1 wait)
        psd = psum.tile([128, TW], F32, tag="ps", bufs=8)
        nc.tensor.matmul(psd[0:1, 0:1], lhsT=d1t_sb[:, 0:1], rhs=d2t_sb[:, 0:1],
                         start=True, stop=True)

        ones8 = const.tile([128, 8], F32)
        nc.vector.memset(ones8[:], 1.0)
        # DVE clock sync past the broadcast + packed-input DMAs
        scr1 = const.tile([128, 1], F32)
        nc.vector.tensor_copy(scr1[:], sq2b[:, 0:1])
        scr2 = const.tile([128, 1], F32)
        nc.vector.tensor_copy(scr2[:], sq1_sb[:, 0:1])

        ob = outp.tile([128, 16], U32)          # cols 0-7 d2 bits, 8-15 idx

        for c in range(NCHUNK):
            # whole-chunk vals buffer (all consumers are DVE-local)
            v = valsp.tile([128, B2], F32, tag="v")
            tm = small.tile([128, NT], F32, tag="tm")

            for t in range(NT):
                ps = psum.tile([128, TW], F32, tag="ps", bufs=8)
                nc.tensor.matmul(ps[:], lhsT=d1t_sb[:, c * 128:(c + 1) * 128],
                                 rhs=d2t_sb[:, t * TW:(t + 1) * TW],
                                 start=True, stop=True)
                # v_t = ps + sq2b ; tm[:, t] = min(v_t)
                nc.vector.scalar_tensor_tensor(
                    out=v[:, t * TW:(t + 1) * TW], in0=ps[:], scalar=1.0,
                    in1=sq2b[:, t * TW:(t + 1) * TW],
                    op0=mybir.AluOpType.mult, op1=mybir.AluOpType.add)
                nc.vector.tensor_reduce(tm[:, t:t + 1], v[:, t * TW:(t + 1) * TW],
                                        axis=mybir.AxisListType.X,
                                        op=mybir.AluOpType.min)

            # ---- narrowing: row min, then first position of it in the row ----
            gm = small.tile([128, 1], F32, tag="gm")
            nc.vector.tensor_reduce(gm[:], tm[:], axis=mybir.AxisListType.X,
                                    op=mybir.AluOpType.min)
            gm8 = small.tile([128, 8], F32, tag="gm8")
            nc.vector.tensor_scalar_mul(gm8[:], ones8[:], gm[:, 0:1])
            n8 = small.tile([128, 8], U32, tag="n8")
            nc.vector.max_index(n8[:], gm8[:], v[:])
            nc.vector.tensor_copy(ob[:, 8 + c:9 + c], n8[:, 0:1])

            d2c = small.tile([128, 1], F32, tag="d2c")
            nc.vector.tensor_tensor(d2c[:], gm[:], sq1_sb[:, c:c + 1],
                                    mybir.AluOpType.add)
            nc.vector.tensor_scalar_max(ob[:, c:c + 1].bitcast(F32), d2c[:], EPS)

        # single packed store (out[x*128+p] = ob[p, x])
        nc.sync.dma_start(out_o.rearrange("(x p) -> p x", p=128), ob[:])

    nc.finalize()
    return nc


def _get_nc():
    global _BUILT
    if _BUILT is None:
        _BUILT = _build()
    return _BUILT


def _shard_inputs(desc1: np.ndarray, desc2: np.ndarray):
    desc1 = np.ascontiguousarray(np.asarray(desc1, dtype=np.float32))
    desc2 = np.ascontiguousarray(np.asarray(desc2, dtype=np.float32))
    d2tt = desc2.T
    sq2 = (desc2 * desc2).sum(axis=1, dtype=np.float32)
    sq1 = (desc1 * desc1).sum(axis=1, dtype=np.float32)
    in_maps = []
    for c in range(NCORES):
        s = desc1[c * B1S:(c + 1) * B1S]
        dxa = np.empty((D, DX), dtype=np.float32)
        dxa[:, :B2] = d2tt
        dxa[:, B2:B2 + B1S] = (-2.0 * s).T
        # sq1 layout [p, chunk]: row m = chunk*128 + p
        dxa[:, B2 + B1S:B2 + B1S + NCHUNK] = \
            sq1[c * B1S:(c + 1) * B1S].reshape(NCHUNK, 128).T
        dxa[:, B2 + B1S + NCHUNK:] = sq2[None, :]
        in_maps.append({"dx": dxa})
    return in_maps


def run_sharded(desc1, desc2, trace=False, **kwargs):
    nc = _get_nc()
    in_maps = _shard_inputs(desc1, desc2)
    res = run_bass_kernel_spmd(nc, in_maps, core_ids=list(range(NCORES)),
                               trace=trace, **kwargs)
    return res


def _assemble(results):
    dists = []
    idxs = []
    for r in results:
        arr = np.asarray(r["out"]).reshape(2 * B1S)
        d2v = arr[:B1S].view(np.float32)
        dists.append(np.sqrt(d2v))  # d2 already clamped at EPS on device
        idxs.append(arr[B1S:].astype(np.int64))
    match_dists = np.concatenate(dists).astype(np.float32).reshape(B1, 1)
    nn = np.concatenate(idxs)
    idxs1 = np.arange(B1, dtype=np.int32)
    matches = np.stack([idxs1, nn.astype(np.int32)], axis=1)
    return match_dists, matches


def kernel(desc1, desc2):
    res = run_sharded(desc1, desc2)
    return _assemble(res.results)


# revision 38
# speedup vs baseline: 1.0273x; 1.0273x over previous
"""Trainium2 Bass kernel for nearest-neighbor descriptor matching (retrieval_knn).

Problem: desc1 (8192, 128), desc2 (8192, 128) fp32. For each row of desc1 find
the nearest row of desc2 under Euclidean distance. Returns
(match_dists (8192, 1) f32, matches_idxs (8192, 2) int32) like the reference.

Strategy (data-parallel over desc1 rows, 8 NeuronCores):
  - each core owns a 1024-row strip of desc1 and a full replica of desc2
  - distances via the norm expansion: d2 = sq1[m] + sq2[n] - 2 * <a_m, b_n>
  - per 128-row chunk (8 chunks): 16 matmuls (K=128, N=512) write -2*cross to
    PSUM; DVE adds the (pre-broadcast) sq2 row into a whole-chunk SBUF vals
    buffer and reduces each 512-wide tile to its min
  - row min via a reduce over the 16 tile-mins; argmin via one max_index
    scan of the row for the min value (first occurrence == np.argmin)
  - device returns clamped squared distances; the final elementwise
    np.sqrt happens on the host (correctly-rounded, matches the reference)

The engine-instruction encodings on this path carry at most ONE semaphore
wait each (including the kernel-tail drain, which waits once per busy
processor), so the kernel is shaped to minimize distinct processors:
one packed input DMA + one broadcast DMA + one packed output DMA, PE + DVE
only, with dummy clock-sync ops so no instruction ever needs two waits.

Inputs per core:
  dx [128, 9288] f32: cols 0..8191 desc2.T; 8192..9215 (-2*desc1 shard).T;
      9216..9223 sq1 in [p, chunk] layout; 9224..9287 sq2 reshaped (128, 64)
Output per core: out [2048] u32: first 1024 = clamped d2 bits (f32),
  last 1024 = argmin indices.

Self-contained: shapes/sharding hardcoded; no file reads.
"""

import os

import numpy as np

import concourse.bacc as bacc
import concourse.bass as bass
import concourse.mybir as mybir
import concourse.tile as tile
from concourse.bass_utils import run_bass_kernel_spmd

# ---- problem constants (hardcoded per contract) ----
B1, B2, D = 8192, 8192, 128
NCORES = 8
B1S = B1 // NCORES          # rows of desc1 per core = 1024
NCHUNK = B1S // 128         # 128-row chunks per core = 8
NT = B2 // 512              # 512-wide tiles per row = 16
TW = 512                    # tile width (one PSUM bank of fp32)
DX = B2 + B1S + NCHUNK + B2          # 8192 + 1024 + 8 + 8192 = 17416
EPS = 1e-12

F32 = mybir.dt.float32
U32 = mybir.dt.uint32

_BUILT = None  # cached Bass program


def _build():
    from contextlib import ExitStack

    nc = bacc.Bacc("TRN2", target_bir_lowering=False)

    dx = nc.dram_tensor("dx", [D, DX], F32, kind="ExternalInput")
    out_o = nc.dram_tensor("out", [2 * B1S], U32, kind="ExternalOutput")

    with ExitStack() as ctx:
        tc = ctx.enter_context(tile.TileContext(nc))
        const = ctx.enter_context(tc.tile_pool(name="const", bufs=1))
        small = ctx.enter_context(tc.tile_pool(name="small", bufs=8))
        valsp = ctx.enter_context(tc.tile_pool(name="valsp", bufs=2))
        psum = ctx.enter_context(tc.tile_pool(name="psum", bufs=8, space="PSUM"))
        outp = ctx.enter_context(tc.tile_pool(name="outp", bufs=1))

        # ---- prologue: one packed load + one broadcast load ----
        dx_sb = const.tile([D, DX], F32)
        nc.sync.dma_start(dx_sb[:], dx[:])

        d2t_sb = dx_sb[:, 0:B2]
        sq2b = dx_sb[:, B2 + B1S + NCHUNK:]
        d1t_sb = dx_sb[:, B2:B2 + B1S]
        sq1_sb = dx_sb[:, B2 + B1S:B2 + B1S + NCHUNK]

        # PE clock sync past the packed-input DMA (matmuls carry 1 wait)
        psd = psum.tile([128, TW], F32, tag="ps", bufs=8)
        nc.tensor.matmul(psd[0:1, 0:1], lhsT=d1t_sb[:, 0:1], rhs=d2t_sb[:, 0:1],
                         start=True, stop=True)

        ones8 = const.tile([128, 8], F32)
        nc.vector.memset(ones8[:], 1.0)
        # DVE clock sync past the broadcast + packed-input DMAs
        scr1 = const.tile([128, 1], F32)
        nc.vector.tensor_copy(scr1[:], sq2b[:, 0:1])
        scr2 = const.tile([128, 1], F32)
        nc.vector.tensor_copy(scr2[:], sq1_sb[:, 0:1])

        ob = outp.tile([128, 16], U32)          # cols 0-7 d2 bits, 8-15 idx

        for c in range(NCHUNK):
            # whole-chunk vals buffer
            v = valsp.tile([128, B2], F32, tag="v")
            tm = small.tile([128, NT], F32, tag="tm")

            for t in range(NT):
                ps = psum.tile([128, TW], F32, tag="ps", bufs=8)
                nc.tensor.matmul(ps[:], lhsT=d1t_sb[:, c * 128:(c + 1) * 128],
                                 rhs=d2t_sb[:, t * TW:(t + 1) * TW],
                                 start=True, stop=True)
                # v_t = ps + sq2b  (the only engine that can read PSUM 2-input)
                nc.vector.scalar_tensor_tensor(
                    out=v[:, t * TW:(t + 1) * TW], in0=ps[:], scalar=1.0,
                    in1=sq2b[:, t * TW:(t + 1) * TW],
                    op0=mybir.AluOpType.mult, op1=mybir.AluOpType.add)

            # ---- narrowing: per-tile mins in one wide reduce, then row min,
            # then first position of the row min in the row ----
            nc.vector.tensor_reduce(tm[:], v.rearrange("p (t w) -> p t w", t=NT),
                                    axis=mybir.AxisListType.X,
                                    op=mybir.AluOpType.min)
            gm = small.tile([128, 1], F32, tag="gm")
            nc.vector.tensor_reduce(gm[:], tm[:], axis=mybir.AxisListType.X,
                                    op=mybir.AluOpType.min)
            gm8 = small.tile([128, 8], F32, tag="gm8")
            nc.vector.tensor_scalar_mul(gm8[:], ones8[:], gm[:, 0:1])
            n8 = small.tile([128, 8], U32, tag="n8")
            nc.vector.max_index(n8[:], gm8[:], v[:])
            nc.vector.tensor_copy(ob[:, 8 + c:9 + c], n8[:, 0:1])

            d2c = small.tile([128, 1], F32, tag="d2c")
            nc.vector.tensor_tensor(d2c[:], gm[:], sq1_sb[:, c:c + 1],
                                    mybir.AluOpType.add)
            nc.vector.tensor_scalar_max(ob[:, c:c + 1].bitcast(F32), d2c[:], EPS)

        # single packed store (out[x*128+p] = ob[p, x])
        nc.sync.dma_start(out_o.rearrange("(x p) -> p x", p=128), ob[:])

    nc.finalize()
    return nc


def _get_nc():
    global _BUILT
    if _BUILT is None:
        _BUILT = _build()
    return _BUILT


def _shard_inputs(desc1: np.ndarray, desc2: np.ndarray):
    desc1 = np.ascontiguousarray(np.asarray(desc1, dtype=np.float32))
    desc2 = np.ascontiguousarray(np.asarray(desc2, dtype=np.float32))
    d2tt = desc2.T
    sq2 = (desc2 * desc2).sum(axis=1, dtype=np.float32)
    sq1 = (desc1 * desc1).sum(axis=1, dtype=np.float32)
    in_maps = []
    for c in range(NCORES):
        s = desc1[c * B1S:(c + 1) * B1S]
        dxa = np.empty((D, DX), dtype=np.float32)
        dxa[:, :B2] = d2tt
        dxa[:, B2:B2 + B1S] = (-2.0 * s).T
        # sq1 layout [p, chunk]: row m = chunk*128 + p
        dxa[:, B2 + B1S:B2 + B1S + NCHUNK] = \
            sq1[c * B1S:(c + 1) * B1S].reshape(NCHUNK, 128).T
        dxa[:, B2 + B1S + NCHUNK:] = sq2[None, :]
        in_maps.append({"dx": dxa})
    return in_maps


def run_sharded(desc1, desc2, trace=False, **kwargs):
    nc = _get_nc()
    in_maps = _shard_inputs(desc1, desc2)
    res = run_bass_kernel_spmd(nc, in_maps, core_ids=list(range(NCORES)),
                               trace=trace, **kwargs)
    return res


def _assemble(results):
    dists = []
    idxs = []
    for r in results:
        arr = np.asarray(r["out"]).reshape(2 * B1S)
        d2v = arr[:B1S].view(np.float32)
        dists.append(np.sqrt(d2v))  # d2 already clamped at EPS on device
        idxs.append(arr[B1S:].astype(np.int64))
    match_dists = np.concatenate(dists).astype(np.float32).reshape(B1, 1)
    nn = np.concatenate(idxs)
    idxs1 = np.arange(B1, dtype=np.int32)
    matches = np.stack([idxs1, nn.astype(np.int32)], axis=1)
    return match_dists, matches


def kernel(desc1, desc2):
    res = run_sharded(desc1, desc2)
    return _assemble(res.results)


# revision 40
# speedup vs baseline: 1.0872x; 1.0583x over previous
"""Trainium2 Bass kernel for nearest-neighbor descriptor matching (retrieval_knn).

Problem: desc1 (8192, 128), desc2 (8192, 128) fp32. For each row of desc1 find
the nearest row of desc2 under Euclidean distance. Returns
(match_dists (8192, 1) f32, matches_idxs (8192, 2) int32) like the reference.

Strategy (data-parallel over desc1 rows, 8 NeuronCores):
  - each core owns a 1024-row strip of desc1 and a full replica of desc2
  - distances via the norm expansion: d2 = sq1[m] + sq2[n] - 2 * <a_m, b_n>
  - per 128-row chunk (8 chunks): 16 matmuls (K=128, N=512) write -2*cross to
    PSUM; DVE adds the (pre-broadcast) sq2 row into a whole-chunk SBUF vals
    buffer (scalar_tensor_tensor, the only 2-input op that can read PSUM)
  - one wide tensor_reduce gives the 16 tile mins, a second gives the row
    min; argmin via one max_index scan of the row for the min value
    (first-occurrence semantics == np.argmin)
  - device returns clamped squared distances; the final elementwise
    np.sqrt happens on the host (correctly-rounded, matches the reference)

The engine-instruction encodings on this path carry at most ONE semaphore
wait each (including the kernel-tail drain, which waits once per busy
processor), so the kernel is shaped to minimize distinct processors:
one packed input DMA + one broadcast DMA + one packed output DMA, PE + DVE
only, with dummy clock-sync ops so no instruction ever needs two waits.

Inputs per core:
  dx [128, 9288] f32: cols 0..8191 desc2.T; 8192..9215 (-2*desc1 shard).T;
      9216..9223 sq1 in [p, chunk] layout; 9224..9287 sq2 reshaped (128, 64)
Output per core: out [2048] u32: first 1024 = clamped d2 bits (f32),
  last 1024 = argmin indices.

Self-contained: shapes/sharding hardcoded; no file reads.
"""

import os

import numpy as np

import concourse.bacc as bacc
import concourse.bass as bass
import concourse.mybir as mybir
import concourse.tile as tile
from concourse.bass_utils import run_bass_kernel_spmd

# ---- problem constants (hardcoded per contract) ----
B1, B2, D = 8192, 8192, 128
NCORES = 8
B1S = B1 // NCORES          # rows of desc1 per core = 1024
NCHUNK = B1S // 128         # 128-row chunks per core = 8
NT = B2 // 512              # 512-wide tiles per row = 16
TW = 512                    # tile width (one PSUM bank of fp32)
DX = B2 + B1S + NCHUNK + B2          # 8192 + 1024 + 8 + 8192 = 17416
EPS = 1e-12

F32 = mybir.dt.float32
U32 = mybir.dt.uint32

_BUILT = None  # cached Bass program


def _build():
    from contextlib import ExitStack

    nc = bacc.Bacc("TRN2", target_bir_lowering=False)

    dx = nc.dram_tensor("dx", [D, DX], F32, kind="ExternalInput")
    out_o = nc.dram_tensor("out", [2 * B1S], U32, kind="ExternalOutput")

    with ExitStack() as ctx:
        tc = ctx.enter_context(tile.TileContext(nc))
        const = ctx.enter_context(tc.tile_pool(name="const", bufs=1))
        small = ctx.enter_context(tc.tile_pool(name="small", bufs=8))
        valsp = ctx.enter_context(tc.tile_pool(name="valsp", bufs=2))
        psum = ctx.enter_context(tc.tile_pool(name="psum", bufs=8, space="PSUM"))
        outp = ctx.enter_context(tc.tile_pool(name="outp", bufs=1))

        # ---- prologue: packed load, split and ordered by first use so the
        # first chunk's matmuls/adds start ~1MB in instead of 8.9MB in ----
        dx_sb = const.tile([D, DX], F32)
        SQ2B0 = B2 + B1S + NCHUNK
        # d1t + sq1 (needed by every matmul's stationary operand)
        nc.sync.dma_start(dx_sb[:, B2:SQ2B0], dx[:, B2:SQ2B0])
        # d2t and sq2b quarter-slices, interleaved in consumption order
        for q in range(4):
            nc.sync.dma_start(dx_sb[:, q * 2048:(q + 1) * 2048],
                              dx[:, q * 2048:(q + 1) * 2048])
            nc.sync.dma_start(dx_sb[:, SQ2B0 + q * 2048:SQ2B0 + (q + 1) * 2048],
                              dx[:, SQ2B0 + q * 2048:SQ2B0 + (q + 1) * 2048])

        d2t_sb = dx_sb[:, 0:B2]
        sq2b = dx_sb[:, B2 + B1S + NCHUNK:]
        d1t_sb = dx_sb[:, B2:B2 + B1S]
        sq1_sb = dx_sb[:, B2 + B1S:B2 + B1S + NCHUNK]

        # PE clock sync past the packed-input DMA (matmuls carry 1 wait)
        psd = psum.tile([128, TW], F32, tag="ps", bufs=8)
        nc.tensor.matmul(psd[0:1, 0:1], lhsT=d1t_sb[:, 0:1], rhs=d2t_sb[:, 0:1],
                         start=True, stop=True)

        ones8 = const.tile([128, 8], F32)
        nc.vector.memset(ones8[:], 1.0)
        # DVE clock sync past the broadcast + packed-input DMAs
        scr1 = const.tile([128, 1], F32)
        nc.vector.tensor_copy(scr1[:], sq2b[:, 0:1])
        scr2 = const.tile([128, 1], F32)
        nc.vector.tensor_copy(scr2[:], sq1_sb[:, 0:1])

        ob = outp.tile([128, 16], U32)          # cols 0-7 d2 bits, 8-15 idx

        for c in range(NCHUNK):
            # whole-chunk vals buffer
            v = valsp.tile([128, B2], F32, tag="v")
            tm = small.tile([128, NT], F32, tag="tm")

            for t in range(NT):
                ps = psum.tile([128, TW], F32, tag="ps", bufs=8)
                nc.tensor.matmul(ps[:], lhsT=d1t_sb[:, c * 128:(c + 1) * 128],
                                 rhs=d2t_sb[:, t * TW:(t + 1) * TW],
                                 start=True, stop=True)
                # v_t = ps + sq2b  (the only engine that can read PSUM 2-input)
                nc.vector.scalar_tensor_tensor(
                    out=v[:, t * TW:(t + 1) * TW], in0=ps[:], scalar=1.0,
                    in1=sq2b[:, t * TW:(t + 1) * TW],
                    op0=mybir.AluOpType.mult, op1=mybir.AluOpType.add)

            # ---- narrowing: per-tile mins in one wide reduce, then row min,
            # then first position of the row min in the row ----
            nc.vector.tensor_reduce(tm[:], v.rearrange("p (t w) -> p t w", t=NT),
                                    axis=mybir.AxisListType.X,
                                    op=mybir.AluOpType.min)
            gm = small.tile([128, 1], F32, tag="gm")
            nc.vector.tensor_reduce(gm[:], tm[:], axis=mybir.AxisListType.X,
                                    op=mybir.AluOpType.min)
            gm8 = small.tile([128, 8], F32, tag="gm8")
            nc.vector.tensor_scalar_mul(gm8[:], ones8[:], gm[:, 0:1])
            n8 = small.tile([128, 8], U32, tag="n8")
            nc.vector.max_index(n8[:], gm8[:], v[:])
            nc.vector.tensor_copy(ob[:, 8 + c:9 + c], n8[:, 0:1])

            d2c = small.tile([128, 1], F32, tag="d2c")
            nc.vector.tensor_tensor(d2c[:], gm[:], sq1_sb[:, c:c + 1],
                                    mybir.AluOpType.add)
            nc.vector.tensor_scalar_max(ob[:, c:c + 1].bitcast(F32), d2c[:], EPS)

        # single packed store (out[x*128+p] = ob[p, x])
        nc.sync.dma_start(out_o.rearrange("(x p) -> p x", p=128), ob[:])

    nc.finalize()
    return nc


def _get_nc():
    global _BUILT
    if _BUILT is None:
        _BUILT = _build()
    return _BUILT


def _shard_inputs(desc1: np.ndarray, desc2: np.ndarray):
    desc1 = np.ascontiguousarray(np.asarray(desc1, dtype=np.float32))
    desc2 = np.ascontiguousarray(np.asarray(desc2, dtype=np.float32))
    d2tt = desc2.T
    sq2 = (desc2 * desc2).sum(axis=1, dtype=np.float32)
    sq1 = (desc1 * desc1).sum(axis=1, dtype=np.float32)
    in_maps = []
    for c in range(NCORES):
        s = desc1[c * B1S:(c + 1) * B1S]
        dxa = np.empty((D, DX), dtype=np.float32)
        dxa[:, :B2] = d2tt
        dxa[:, B2:B2 + B1S] = (-2.0 * s).T
        # sq1 layout [p, chunk]: row m = chunk*128 + p
        dxa[:, B2 + B1S:B2 + B1S + NCHUNK] = \
            sq1[c * B1S:(c + 1) * B1S].reshape(NCHUNK, 128).T
        dxa[:, B2 + B1S + NCHUNK:] = sq2[None, :]
        in_maps.append({"dx": dxa})
    return in_maps


def run_sharded(desc1, desc2, trace=False, **kwargs):
    nc = _get_nc()
    in_maps = _shard_inputs(desc1, desc2)
    res = run_bass_kernel_spmd(nc, in_maps, core_ids=list(range(NCORES)),
                               trace=trace, **kwargs)
    return res


def _assemble(results):
    dists = []
    idxs = []
    for r in results:
        arr = np.asarray(r["out"]).reshape(2 * B1S)
        d2v = arr[:B1S].view(np.float32)
        dists.append(np.sqrt(d2v))  # d2 already clamped at EPS on device
        idxs.append(arr[B1S:].astype(np.int64))
    match_dists = np.concatenate(dists).astype(np.float32).reshape(B1, 1)
    nn = np.concatenate(idxs)
    idxs1 = np.arange(B1, dtype=np.int32)
    matches = np.stack([idxs1, nn.astype(np.int32)], axis=1)
    return match_dists, matches


def kernel(desc1, desc2):
    res = run_sharded(desc1, desc2)
    return _assemble(res.results)


# revision 42
# speedup vs baseline: 1.1161x; 1.0266x over previous
"""Trainium2 Bass kernel for nearest-neighbor descriptor matching (retrieval_knn).

Problem: desc1 (8192, 128), desc2 (8192, 128) fp32. For each row of desc1 find
the nearest row of desc2 under Euclidean distance. Returns
(match_dists (8192, 1) f32, matches_idxs (8192, 2) int32) like the reference.

Strategy (data-parallel over desc1 rows, 8 NeuronCores):
  - each core owns a 1024-row strip of desc1 and a full replica of desc2
  - distances via the norm expansion: d2 = sq1[m] + sq2[n] - 2 * <a_m, b_n>
  - per 128-row chunk (8 chunks): 16 matmuls (K=128, N=512) write -2*cross to
    PSUM; DVE adds the (pre-broadcast) sq2 row into a whole-chunk SBUF vals
    buffer (scalar_tensor_tensor, the only 2-input op that can read PSUM)
  - one wide tensor_reduce gives the 16 tile mins, a second gives the row
    min; argmin via one max_index scan of the row for the min value
    (first-occurrence semantics == np.argmin)
  - device returns clamped squared distances; the final elementwise
    np.sqrt happens on the host (correctly-rounded, matches the reference)

The engine-instruction encodings on this path carry at most ONE semaphore
wait each (including the kernel-tail drain, which waits once per busy
processor), so the kernel is shaped to minimize distinct processors:
one packed input DMA + one broadcast DMA + one packed output DMA, PE + DVE
only, with dummy clock-sync ops so no instruction ever needs two waits.

Inputs per core:
  dx [128, 9288] f32: cols 0..8191 desc2.T; 8192..9215 (-2*desc1 shard).T;
      9216..9223 sq1 in [p, chunk] layout; 9224..9287 sq2 reshaped (128, 64)
Output per core: out [2048] u32: first 1024 = clamped d2 bits (f32),
  last 1024 = argmin indices.

Self-contained: shapes/sharding hardcoded; no file reads.
"""

import os

import numpy as np

import concourse.bacc as bacc
import concourse.bass as bass
import concourse.mybir as mybir
import concourse.tile as tile
from concourse.bass_utils import run_bass_kernel_spmd

# ---- problem constants (hardcoded per contract) ----
B1, B2, D = 8192, 8192, 128
NCORES = 8
B1S = B1 // NCORES          # rows of desc1 per core = 1024
NCHUNK = B1S // 128         # 128-row chunks per core = 8
NT = B2 // 512              # 512-wide tiles per row = 16
TW = 512                    # tile width (one PSUM bank of fp32)
DX = B2 + B1S + NCHUNK + B2          # 8192 + 1024 + 8 + 8192 = 17416
EPS = 1e-12

F32 = mybir.dt.float32
U32 = mybir.dt.uint32

_BUILT = None  # cached Bass program


def _build():
    from contextlib import ExitStack

    nc = bacc.Bacc("TRN2", target_bir_lowering=False)

    dx = nc.dram_tensor("dx", [D, DX], F32, kind="ExternalInput")
    out_o = nc.dram_tensor("out", [2 * B1S], U32, kind="ExternalOutput")

    with ExitStack() as ctx:
        tc = ctx.enter_context(tile.TileContext(nc))
        const = ctx.enter_context(tc.tile_pool(name="const", bufs=1))
        small = ctx.enter_context(tc.tile_pool(name="small", bufs=8))
        valsp = ctx.enter_context(tc.tile_pool(name="valsp", bufs=2))
        psum = ctx.enter_context(tc.tile_pool(name="psum", bufs=8, space="PSUM"))
        outp = ctx.enter_context(tc.tile_pool(name="outp", bufs=1))

        # ---- prologue: packed load, split and ordered by first use so the
        # first chunk's matmuls/adds start ~1MB in instead of 8.9MB in ----
        dx_sb = const.tile([D, DX], F32)
        SQ2B0 = B2 + B1S + NCHUNK
        # d1t + sq1 (needed by every matmul's stationary operand)
        nc.sync.dma_start(dx_sb[:, B2:SQ2B0], dx[:, B2:SQ2B0])
        # d2t and sq2b quarter-slices, interleaved in consumption order
        for q in range(4):
            nc.sync.dma_start(dx_sb[:, q * 2048:(q + 1) * 2048],
                              dx[:, q * 2048:(q + 1) * 2048])
            nc.sync.dma_start(dx_sb[:, SQ2B0 + q * 2048:SQ2B0 + (q + 1) * 2048],
                              dx[:, SQ2B0 + q * 2048:SQ2B0 + (q + 1) * 2048])

        d2t_sb = dx_sb[:, 0:B2]
        sq2b = dx_sb[:, B2 + B1S + NCHUNK:]
        d1t_sb = dx_sb[:, B2:B2 + B1S]
        sq1_sb = dx_sb[:, B2 + B1S:B2 + B1S + NCHUNK]

        ones8 = const.tile([128, 8], F32)
        nc.vector.memset(ones8[:], 1.0)

        ob = outp.tile([128, 16], U32)          # cols 0-7 d2 bits, 8-15 idx

        for c in range(NCHUNK):
            # whole-chunk vals buffer
            v = valsp.tile([128, B2], F32, tag="v")
            tm = small.tile([128, NT], F32, tag="tm")

            for t in range(NT):
                ps = psum.tile([128, TW], F32, tag="ps", bufs=8)
                nc.tensor.matmul(ps[:], lhsT=d1t_sb[:, c * 128:(c + 1) * 128],
                                 rhs=d2t_sb[:, t * TW:(t + 1) * TW],
                                 start=True, stop=True)
                # v_t = ps + sq2b  (the only engine that can read PSUM 2-input)
                nc.vector.scalar_tensor_tensor(
                    out=v[:, t * TW:(t + 1) * TW], in0=ps[:], scalar=1.0,
                    in1=sq2b[:, t * TW:(t + 1) * TW],
                    op0=mybir.AluOpType.mult, op1=mybir.AluOpType.add)
                if t % 4 == 3:
                    # per-quarter tile-min reduce: starts while later input
                    # quarters are still streaming in
                    q = t // 4
                    nc.vector.tensor_reduce(
                        tm[:, 4 * q:4 * q + 4],
                        v[:, 4 * q * TW:(4 * q + 4) * TW].rearrange(
                            "p (t w) -> p t w", t=4),
                        axis=mybir.AxisListType.X, op=mybir.AluOpType.min)

            # ---- narrowing: row min, then its first position in the row ----
            gm = small.tile([128, 1], F32, tag="gm")
            nc.vector.tensor_reduce(gm[:], tm[:], axis=mybir.AxisListType.X,
                                    op=mybir.AluOpType.min)
            gm8 = small.tile([128, 8], F32, tag="gm8")
            nc.vector.tensor_scalar_mul(gm8[:], ones8[:], gm[:, 0:1])
            n8 = small.tile([128, 8], U32, tag="n8")
            nc.vector.max_index(n8[:], gm8[:], v[:])
            nc.vector.tensor_copy(ob[:, 8 + c:9 + c], n8[:, 0:1])

            d2c = small.tile([128, 1], F32, tag="d2c")
            nc.vector.tensor_tensor(d2c[:], gm[:], sq1_sb[:, c:c + 1],
                                    mybir.AluOpType.add)
            nc.vector.tensor_scalar_max(ob[:, c:c + 1].bitcast(F32), d2c[:], EPS)

        # single packed store (out[x*128+p] = ob[p, x])
        nc.sync.dma_start(out_o.rearrange("(x p) -> p x", p=128), ob[:])

    nc.finalize()
    return nc


def _get_nc():
    global _BUILT
    if _BUILT is None:
        _BUILT = _build()
    return _BUILT


def _shard_inputs(desc1: np.ndarray, desc2: np.ndarray):
    desc1 = np.ascontiguousarray(np.asarray(desc1, dtype=np.float32))
    desc2 = np.ascontiguousarray(np.asarray(desc2, dtype=np.float32))
    d2tt = desc2.T
    sq2 = (desc2 * desc2).sum(axis=1, dtype=np.float32)
    sq1 = (desc1 * desc1).sum(axis=1, dtype=np.float32)
    in_maps = []
    for c in range(NCORES):
        s = desc1[c * B1S:(c + 1) * B1S]
        dxa = np.empty((D, DX), dtype=np.float32)
        dxa[:, :B2] = d2tt
        dxa[:, B2:B2 + B1S] = (-2.0 * s).T
        # sq1 layout [p, chunk]: row m = chunk*128 + p
        dxa[:, B2 + B1S:B2 + B1S + NCHUNK] = \
            sq1[c * B1S:(c + 1) * B1S].reshape(NCHUNK, 128).T
        dxa[:, B2 + B1S + NCHUNK:] = sq2[None, :]
        in_maps.append({"dx": dxa})
    return in_maps


def run_sharded(desc1, desc2, trace=False, **kwargs):
    nc = _get_nc()
    in_maps = _shard_inputs(desc1, desc2)
    res = run_bass_kernel_spmd(nc, in_maps, core_ids=list(range(NCORES)),
                               trace=trace, **kwargs)
    return res


def _assemble(results):
    dists = []
    idxs = []
    for r in results:
        arr = np.asarray(r["out"]).reshape(2 * B1S)
        d2v = arr[:B1S].view(np.float32)
        dists.append(np.sqrt(d2v))  # d2 already clamped at EPS on device
        idxs.append(arr[B1S:].astype(np.int64))
    match_dists = np.concatenate(dists).astype(np.float32).reshape(B1, 1)
    nn = np.concatenate(idxs)
    idxs1 = np.arange(B1, dtype=np.int32)
    matches = np.stack([idxs1, nn.astype(np.int32)], axis=1)
    return match_dists, matches


def kernel(desc1, desc2):
    res = run_sharded(desc1, desc2)
    return _assemble(res.results)
